# revision 9
# baseline (speedup 1.0000x reference)
"""Trainium2 Bass kernel for nn_DifferentiablePruner (gnn_message_passing).

Math (see reference):
    i, j = edge_index
    feat = concat(x[i], x[j])           # [E, 2D]
    h    = relu(feat @ W1 + b1)         # [E, H]
    sim  = h @ W2 + b2                  # [E]
    edge_gates = sim * sigmoid((ela + log(u) - log1p(-u)) / beta)
    wg1  = sigmoid((wla1 + log(u_w1) - log1p(-u_w1)) / beta)
    wg2  = sigmoid((wla2 + log(u_w2) - log1p(-u_w2)) / beta)

Key algebraic restructuring: feat @ W1 = x[i] @ W1[:D] + x[j] @ W1[D:], so we
precompute per-node projections P' = x @ W1[:D] + b1 and Q = x @ W1[D:]
([N, H] each, tiny), pack them as fp16 rows R[n] = [P'[n] | Q[n]] (256 B), and
per edge gather two 256 B rows instead of two 1 KB x-rows (4x less traffic).

Sharding: edges are sharded across the 8 cores (spec hint); x / MLP weights
replicated; wg1/wg2 row-sharded across cores.

Per-core pipeline:
  1. PE: transpose x chunks, matmul -> P'|Q, pack fp16 R table in DRAM.
  2. dma_gather (custom SWDGE gather, transpose mode) pulls R[i] and R[j+half]
     rows transposed so features land on partitions: g_i[0:64,e] = P'[i_e],
     g_j[0:64,e] = Q[j_e].
  3. DVE: t = g_i + g_j, h = max(t, 0)  (fp16, feature-on-partition)
  4. PE: sim rows = W2^T @ h  (K=64, M=1, N=320 matmuls into per-block PSUM rows)
  5. ACT/DVE: concrete-gate math in fp32 on [128, 320] tiles, natural layout.
"""

import sys

sys.path.insert(0, "/opt/trn_rl_repo")

import numpy as np

import concourse.bass as bass
import concourse.tile as tile
from concourse import mybir
from concourse.library_config import mlp as MLP_LIB
from concourse.library_overlay import lower_extended_insts

F32 = mybir.dt.float32
F16 = mybir.dt.float16
I16 = mybir.dt.int16
AF = mybir.ActivationFunctionType
ALU = mybir.AluOpType

P = 128  # partitions


class Cfg:
    def __init__(self, n_nodes, epad, BG, n_cores, D=256, H=64, beta=0.1,
                 wg1_rows=128, wg1_cols=256, wg2_rows=128, wg2_cols=128):
        assert epad % BG == 0 and BG % P == 0
        self.N = n_nodes
        self.D = D
        self.H = H
        self.beta = beta
        self.EPAD = epad                      # padded edges per core
        self.BG = BG                          # gather batch (edges per dma_gather)
        self.NBLK = epad // P                 # sim PSUM columns (blocks of 128 edges)
        self.NBATCH = epad // BG
        self.BLK_PER_BATCH = BG // P
        self.SI = epad // 16                  # idx columns (16-way wrapped)
        self.n_cores = n_cores
        self.NCHUNK = (n_nodes + P - 1) // P  # node chunks for precompute
        self.NPAD = self.NCHUNK * P
        self.RROWS = self.NPAD + P            # +pad row(s) for the offset gather
        # per-core weight-gate chunk shapes ([p, cols])
        self.wg1_rows, self.wg1_cols = wg1_rows, wg1_cols
        self.wg2_rows, self.wg2_cols = wg2_rows, wg2_cols
        assert self.NBLK * 4 <= 2048  # one PSUM bank


def split_sync_waits(nc, max_waits=1):
    """walrus CTRL codegen accepts at most one sync-wait per instruction;
    move extras onto preceding same-engine NOPs."""
    ctr = 0
    for fn in nc.m.functions:
        for bb in fn.blocks:
            newlist = []
            for ins in bb.instructions:
                si = ins.sync_info
                if si is not None and si.on_wait and len(si.on_wait) > max_waits:
                    waits = list(si.on_wait)
                    keep = waits[-max_waits:]
                    extra = waits[:-max_waits]
                    for k in range(0, len(extra), max_waits):
                        ctr += 1
                        nop = mybir.InstNoOp(name=f"SplitW-{ctr}", ins=[], outs=[])
                        nop.engine = ins.engine
                        nop.sync_info = mybir.SyncInfo(
                            on_wait=extra[k:k + max_waits], on_update=[])
                        newlist.append(nop)
                    ins.sync_info = mybir.SyncInfo(
                        on_wait=keep, on_update=list(si.on_update))
                newlist.append(ins)
            bb.instructions = newlist
    return ctr


def _gate_chain(nc, pool, la_t, u_t, shape, scale):
    """Return fp32 tile gate = sigmoid((la + log(u) - log1p(-u)) * scale).
    la_t/u_t are fp32 SBUF tiles of `shape`."""
    p, f = shape
    lg_u = pool.tile([p, f], F32, tag="gate_lgu")
    nc.scalar.activation(out=lg_u[:], in_=u_t[:], func=AF.Ln)
    lg_m = pool.tile([p, f], F32, tag="gate_lgm")
    # log(1 - u) = Ln(u * -1 + 1)
    nc.scalar.activation(out=lg_m[:], in_=u_t[:], func=AF.Ln, scale=-1.0, bias=1.0)
    d = pool.tile([p, f], F32, tag="gate_d")
    nc.vector.tensor_tensor(out=d[:], in0=lg_u[:], in1=lg_m[:], op=ALU.subtract)
    arg = pool.tile([p, f], F32, tag="gate_arg")
    nc.vector.tensor_tensor(out=arg[:], in0=d[:], in1=la_t[:], op=ALU.add)
    gate = pool.tile([p, f], F32, tag="gate_out")
    nc.scalar.activation(out=gate[:], in_=arg[:], func=AF.Sigmoid, scale=scale)
    return gate


def build_kernel(cfg: Cfg, split_waits: bool = True):
    nc = bass.Bass("TRN2", target_bir_lowering=False, debug=False,
                   num_devices=cfg.n_cores)
    N, D, H = cfg.N, cfg.D, cfg.H
    KT = D // P  # k-tiles per projection (2 for D=256)

    # ---- I/O ----
    x_in = nc.dram_tensor("x", [N, D], F32, kind="ExternalInput")
    w1_in = nc.dram_tensor("w1", [2 * D, H], F32, kind="ExternalInput")
    b1_in = nc.dram_tensor("b1", [H], F32, kind="ExternalInput")
    w2_in = nc.dram_tensor("w2", [H, 1], F32, kind="ExternalInput")
    b2_in = nc.dram_tensor("b2", [1], F32, kind="ExternalInput")
    ident_in = nc.dram_tensor("ident", [P, P], F32, kind="ExternalInput")
    idx_i_in = nc.dram_tensor("idx_i", [P, cfg.SI], I16, kind="ExternalInput")
    idx_j_in = nc.dram_tensor("idx_j", [P, cfg.SI], I16, kind="ExternalInput")
    ela_in = nc.dram_tensor("ela", [P, cfg.NBLK], F32, kind="ExternalInput")
    u_in = nc.dram_tensor("u", [P, cfg.NBLK], F32, kind="ExternalInput")
    wla1_in = nc.dram_tensor("wla1c", [cfg.wg1_rows, cfg.wg1_cols], F32,
                             kind="ExternalInput")
    uw1_in = nc.dram_tensor("uw1c", [cfg.wg1_rows, cfg.wg1_cols], F32,
                            kind="ExternalInput")
    wla2_in = nc.dram_tensor("wla2c", [cfg.wg2_rows, cfg.wg2_cols], F32,
                             kind="ExternalInput")
    uw2_in = nc.dram_tensor("uw2c", [cfg.wg2_rows, cfg.wg2_cols], F32,
                            kind="ExternalInput")

    eg_out = nc.dram_tensor("eg", [P, cfg.NBLK], F32, kind="ExternalOutput")
    wg1_out = nc.dram_tensor("wg1c", [cfg.wg1_rows, cfg.wg1_cols], F32,
                             kind="ExternalOutput")
    wg2_out = nc.dram_tensor("wg2c", [cfg.wg2_rows, cfg.wg2_cols], F32,
                             kind="ExternalOutput")

    # fp16 node-projection table: row n = [P'(n) | Q(n)], 256 B
    r_dram = nc.dram_tensor("r_table", [cfg.RROWS, 2 * H], F16, kind="Internal")

    inv_beta = 1.0 / cfg.beta

    with tile.TileContext(nc) as tc:
        with (
            tc.tile_pool(name="singles", bufs=1) as singles,
            tc.tile_pool(name="pre", bufs=3) as pre,
            tc.tile_pool(name="pre_ps", bufs=2, space="PSUM") as pre_ps,
            tc.tile_pool(name="pq_ps", bufs=2, space="PSUM") as pq_ps,
            tc.tile_pool(name="gather", bufs=3) as gather,
            tc.tile_pool(name="hwork", bufs=3) as hwork,
            tc.tile_pool(name="sim_ps", bufs=1, space="PSUM") as sim_ps_pool,
            tc.tile_pool(name="gate", bufs=2) as gate_pool,
        ):
            # ---- constants ----
            nc.gpsimd.load_library(MLP_LIB)
            ident = singles.tile([P, P], F32)
            nc.sync.dma_start(out=ident[:], in_=ident_in.ap())
            w1_sb = singles.tile([P, KT, 2, H], F32)
            for kt in range(KT):
                nc.sync.dma_start(
                    out=w1_sb[:, kt, :, :],
                    in_=bass.AP(tensor=w1_in, offset=kt * P * H,
                                ap=[[H, P], [D * H, 2], [1, H]]),
                )
            b1_bc = singles.tile([P, H], F32)
            nc.sync.dma_start(
                out=b1_bc[:],
                in_=bass.AP(tensor=b1_in, offset=0, ap=[[0, P], [1, H]]),
            )
            w2_f32 = singles.tile([H, 1], F32)
            nc.sync.dma_start(out=w2_f32[:], in_=w2_in.ap())
            w2_f16 = singles.tile([H, 1], F16)
            nc.vector.tensor_copy(out=w2_f16[:], in_=w2_f32[:])
            b2_bc = singles.tile([P, 1], F32)
            nc.sync.dma_start(
                out=b2_bc[:],
                in_=bass.AP(tensor=b2_in, offset=0, ap=[[0, P], [1, 1]]),
            )
            idx_i_sb = singles.tile([P, cfg.SI], I16)
            nc.sync.dma_start(out=idx_i_sb[:], in_=idx_i_in.ap())
            idx_j_sb = singles.tile([P, cfg.SI], I16)
            nc.sync.dma_start(out=idx_j_sb[:], in_=idx_j_in.ap())
            w1_f16 = singles.tile([P, KT, 2 * H], F16)
            for kt in range(KT):
                nc.scalar.copy(out=w1_f16[:, kt, :], in_=w1_sb[:, kt, :, :])

            # ---- phase 1: build R table ----
            for c in range(cfg.NCHUNK):
                n0 = c * P
                cnt = min(P, N - n0)
                x_t = pre.tile([P, D], F32, tag="x_t")
                if cnt < P:
                    nc.vector.memset(x_t[:], 0.0)
                nc.sync.dma_start(out=x_t[:cnt, :], in_=x_in.ap()[n0:n0 + cnt, :])

                pq_psum = pq_ps.tile([P, 2 * H], F32, tag="pq")
                for kt in range(KT):
                    xt_ps = pre_ps.tile([P, P], F32, tag="xt_ps")
                    nc.tensor.transpose(
                        out=xt_ps[:], in_=x_t[:, kt * P:(kt + 1) * P],
                        identity=ident[:])
                    xt_sb = pre.tile([P, P], F16, tag="xt_sb")
                    nc.vector.tensor_copy(out=xt_sb[:], in_=xt_ps[:])
                    nc.tensor.matmul(
                        pq_psum[:], lhsT=xt_sb[:], rhs=w1_f16[:, kt, :],
                        start=(kt == 0), stop=(kt == KT - 1))

                r_sb = pre.tile([P, 2 * H], F16, tag="r_sb")
                # P' half: add b1 while casting to fp16
                nc.vector.tensor_tensor(
                    out=r_sb[:, 0:H], in0=pq_psum[:, 0:H], in1=b1_bc[:],
                    op=ALU.add)
                nc.vector.tensor_copy(out=r_sb[:, H:2 * H], in_=pq_psum[:, H:2 * H])
                nc.sync.dma_start(
                    out=r_dram.ap()[n0:n0 + P, :], in_=r_sb[:])

            # zero the pad rows after NPAD (offset gather may touch row NPAD)
            zpad = pre.tile([P, 2 * H], F16, tag="r_sb")
            nc.vector.memset(zpad[:], 0.0)
            nc.sync.dma_start(out=r_dram.ap()[cfg.NPAD:cfg.NPAD + P, :], in_=zpad[:])

            # ---- phase 2: edge pipeline ----
            sim_psum = sim_ps_pool.tile([P, cfg.NBLK], F32)
            gat_src_i = bass.AP(tensor=r_dram, offset=0,
                                ap=[[2 * H, cfg.NPAD], [1, 2 * H]])
            gat_src_j = bass.AP(tensor=r_dram, offset=H,
                                ap=[[2 * H, cfg.NPAD], [1, 2 * H]])

            for bt in range(cfg.NBATCH):
                s0 = bt * (cfg.BG // 16)
                g_i = gather.tile([P, 1, cfg.BG], F16, tag="g_i")
                nc.gpsimd.dma_gather(
                    out_ap=g_i[:], in_ap=gat_src_i,
                    idxs_ap=idx_i_sb[:, s0:s0 + cfg.BG // 16],
                    num_idxs=cfg.BG, num_idxs_reg=cfg.BG,
                    elem_size=2 * H, elem_step=2 * H, transpose=True,
                    single_packet=False)
                g_j = gather.tile([P, 1, cfg.BG], F16, tag="g_j")
                nc.gpsimd.dma_gather(
                    out_ap=g_j[:], in_ap=gat_src_j,
                    idxs_ap=idx_j_sb[:, s0:s0 + cfg.BG // 16],
                    num_idxs=cfg.BG, num_idxs_reg=cfg.BG,
                    elem_size=2 * H, elem_step=2 * H, transpose=True,
                    single_packet=False)

                h_t = hwork.tile([H, cfg.BG], F16, tag="h_t")
                nc.vector.tensor_tensor(
                    out=h_t[:], in0=g_i[0:H, 0, :], in1=g_j[0:H, 0, :],
                    op=ALU.add)
                nc.vector.tensor_scalar_max(out=h_t[:], in0=h_t[:], scalar1=0.0)

                for s in range(cfg.BLK_PER_BATCH):
                    col = bt * cfg.BLK_PER_BATCH + s
                    nc.tensor.matmul(
                        sim_psum[:, col:col + 1],
                        lhsT=h_t[:, s * P:(s + 1) * P], rhs=w2_f16[:],
                        start=True, stop=True)

            # ---- phase 3: edge gates ----
            ela_t = gate_pool.tile([P, cfg.NBLK], F32, tag="ela")
            nc.sync.dma_start(out=ela_t[:], in_=ela_in.ap())
            u_t = gate_pool.tile([P, cfg.NBLK], F32, tag="u")
            nc.sync.dma_start(out=u_t[:], in_=u_in.ap())
            gate = _gate_chain(nc, gate_pool, ela_t, u_t, (P, cfg.NBLK),
                               inv_beta)
            eg_t = gate_pool.tile([P, cfg.NBLK], F32, tag="eg")
            # eg = (sim + b2) * gate
            nc.vector.scalar_tensor_tensor(
                out=eg_t[:], in0=sim_psum[:],
                scalar=b2_bc[:, 0:1], in1=gate[:],
                op0=ALU.add, op1=ALU.mult)
            nc.sync.dma_start(out=eg_out.ap(), in_=eg_t[:])

            # ---- phase 4: weight gates ----
            for la_in, uu_in, oo, rows, cols in (
                (wla1_in, uw1_in, wg1_out, cfg.wg1_rows, cfg.wg1_cols),
                (wla2_in, uw2_in, wg2_out, cfg.wg2_rows, cfg.wg2_cols),
            ):
                la_t = gate_pool.tile([rows, cols], F32, tag="wla")
                nc.sync.dma_start(out=la_t[:], in_=la_in.ap())
                uu_t = gate_pool.tile([rows, cols], F32, tag="wu")
                nc.sync.dma_start(out=uu_t[:], in_=uu_in.ap())
                g_t = _gate_chain(nc, gate_pool, la_t, uu_t, (rows, cols),
                                  inv_beta)
                nc.sync.dma_start(out=oo.ap(), in_=g_t[:])

    lower_extended_insts(nc)
    if split_waits:
        split_sync_waits(nc)
    return nc


# ---------------- host side ----------------

N, E, D, H = 10000, 320000, 256, 64
N_CORES = 8
E_CORE = E // N_CORES            # 40000
REAL_CFG = Cfg(n_nodes=N, epad=40960, BG=5120, n_cores=N_CORES)

_nc_cache = {}


def _get_nc(cfg):
    key = id(cfg)
    if key not in _nc_cache:
        _nc_cache[key] = build_kernel(cfg)
    return _nc_cache[key]


def _wrap_idx(idx, si):
    """int16 indices -> [128, si] wrapped (16 partitions, replicated x8)."""
    idx = idx.astype(np.int16)
    w = idx.reshape(si, 16).T                      # [16, si]
    return np.tile(w, (8, 1)).copy()               # [128, si]


def make_in_maps(cfg, x, edge_index, edge_log_alpha, W1, b1, W2, b2,
                 wla1, wla2, u_edge, u_w1, u_w2):
    n_cores = cfg.n_cores
    e_core = edge_index.shape[1] // n_cores
    wg1_flat = wla1.reshape(n_cores, -1)
    uw1_flat = u_w1.reshape(n_cores, -1)
    wg2_flat = wla2.reshape(n_cores, -1)
    uw2_flat = u_w2.reshape(n_cores, -1)
    in_maps = []
    for c in range(n_cores):
        sl = slice(c * e_core, (c + 1) * e_core)
        ii = np.zeros(cfg.EPAD, np.int64)
        jj = np.zeros(cfg.EPAD, np.int64)
        ii[:e_core] = edge_index[0, sl]
        jj[:e_core] = edge_index[1, sl]
        ela = np.zeros(cfg.EPAD, np.float32)
        ela[:e_core] = edge_log_alpha[sl]
        uu = np.full(cfg.EPAD, 0.5, np.float32)
        uu[:e_core] = u_edge[sl]
        in_maps.append({
            "x": np.ascontiguousarray(x),
            "ident": np.eye(P, dtype=np.float32),
            "w1": np.ascontiguousarray(W1),
            "b1": np.ascontiguousarray(b1),
            "w2": np.ascontiguousarray(W2),
            "b2": np.ascontiguousarray(b2),
            "idx_i": _wrap_idx(ii, cfg.SI),
            "idx_j": _wrap_idx(jj, cfg.SI),
            "ela": np.ascontiguousarray(ela.reshape(cfg.NBLK, P).T),
            "u": np.ascontiguousarray(uu.reshape(cfg.NBLK, P).T),
            "wla1c": wg1_flat[c].reshape(cfg.wg1_rows, cfg.wg1_cols).copy(),
            "uw1c": uw1_flat[c].reshape(cfg.wg1_rows, cfg.wg1_cols).copy(),
            "wla2c": wg2_flat[c].reshape(cfg.wg2_rows, cfg.wg2_cols).copy(),
            "uw2c": uw2_flat[c].reshape(cfg.wg2_rows, cfg.wg2_cols).copy(),
        })
    return in_maps


def assemble(cfg, results, e_total, wla1_shape, wla2_shape):
    n_cores = cfg.n_cores
    e_core = e_total // n_cores
    eg = np.concatenate(
        [r["eg"].T.reshape(-1)[:e_core] for r in results])
    wg1 = np.concatenate([r["wg1c"].reshape(-1) for r in results]).reshape(
        wla1_shape)
    wg2 = np.concatenate([r["wg2c"].reshape(-1) for r in results]).reshape(
        wla2_shape)
    return eg, wg1, wg2


def kernel(x, edge_index, edge_log_alpha, W1, b1, W2, b2, wla1, wla2,
           u_edge, u_w1, u_w2):
    from concourse.bass_utils import run_bass_kernel_spmd

    cfg = REAL_CFG
    nc = _get_nc(cfg)
    in_maps = make_in_maps(
        cfg, np.asarray(x), np.asarray(edge_index),
        np.asarray(edge_log_alpha), np.asarray(W1), np.asarray(b1),
        np.asarray(W2), np.asarray(b2), np.asarray(wla1), np.asarray(wla2),
        np.asarray(u_edge), np.asarray(u_w1), np.asarray(u_w2))
    res = run_bass_kernel_spmd(nc, in_maps, core_ids=list(range(cfg.n_cores)))
    eg, wg1, wg2 = assemble(cfg, res.results, E, wla1.shape, wla2.shape)
    return eg.astype(np.float32), wg1.astype(np.float32), wg2.astype(np.float32)


# revision 13
# speedup vs baseline: 33.4334x; 33.4334x over previous
"""Trainium2 Bass kernel for nn_DifferentiablePruner (gnn_message_passing).

Math (see reference):
    i, j = edge_index
    feat = concat(x[i], x[j])           # [E, 2D]
    h    = relu(feat @ W1 + b1)         # [E, H]
    sim  = h @ W2 + b2                  # [E]
    edge_gates = sim * sigmoid((ela + log(u) - log1p(-u)) / beta)
    wg1  = sigmoid((wla1 + log(u_w1) - log1p(-u_w1)) / beta)
    wg2  = sigmoid((wla2 + log(u_w2) - log1p(-u_w2)) / beta)

Key algebraic restructuring: feat @ W1 = x[i] @ W1[:D] + x[j] @ W1[D:], so we
precompute per-node projections P' = x @ W1[:D] + b1 and Q = x @ W1[D:]
([N, H] each, tiny), pack them as fp16 rows R[n] = [P'[n] | Q[n]] (256 B), and
per edge gather two 256 B rows instead of two 1 KB x-rows (4x less traffic).

Sharding: edges are sharded across the 8 cores (spec hint); x / MLP weights
replicated; wg1/wg2 row-sharded across cores.

Per-core pipeline:
  1. PE: transpose x chunks, matmul -> P'|Q, pack fp16 R table in DRAM.
  2. dma_gather (custom SWDGE gather, transpose mode) pulls R[i] and R[j+half]
     rows transposed so features land on partitions: g_i[0:64,e] = P'[i_e],
     g_j[0:64,e] = Q[j_e].
  3. DVE: t = g_i + g_j, h = max(t, 0)  (fp16, feature-on-partition)
  4. PE: sim rows = W2^T @ h  (K=64, M=1, N=320 matmuls into per-block PSUM rows)
  5. ACT/DVE: concrete-gate math in fp32 on [128, 320] tiles, natural layout.
"""

import sys

sys.path.insert(0, "/opt/trn_rl_repo")

import numpy as np

import concourse.bass as bass
import concourse.tile as tile
from concourse import mybir
from concourse.library_config import mlp as MLP_LIB
from concourse.library_overlay import lower_extended_insts

F32 = mybir.dt.float32
F16 = mybir.dt.float16
I16 = mybir.dt.int16
AF = mybir.ActivationFunctionType
ALU = mybir.AluOpType

P = 128  # partitions


class Cfg:
    def __init__(self, n_nodes, epad, BG, n_cores, D=256, H=64, beta=0.1,
                 wg1_rows=128, wg1_cols=256, wg2_rows=128, wg2_cols=128):
        assert epad % BG == 0 and BG % P == 0
        self.N = n_nodes
        self.D = D
        self.H = H
        self.beta = beta
        self.EPAD = epad                      # padded edges per core
        self.BG = BG                          # gather batch (edges per dma_gather)
        self.NBLK = epad // P                 # sim PSUM columns (blocks of 128 edges)
        self.NBATCH = epad // BG
        self.BLK_PER_BATCH = BG // P
        self.SI = epad // 16                  # idx columns (16-way wrapped)
        self.n_cores = n_cores
        self.NCHUNK = (n_nodes + P - 1) // P  # node chunks for precompute
        self.NPAD = self.NCHUNK * P
        self.RROWS = self.NPAD + P            # +pad row(s) for the offset gather
        # per-core weight-gate chunk shapes ([p, cols])
        self.wg1_rows, self.wg1_cols = wg1_rows, wg1_cols
        self.wg2_rows, self.wg2_cols = wg2_rows, wg2_cols
        assert self.NBLK * 4 <= 2048  # one PSUM bank


def split_sync_waits(nc, max_waits=1):
    """walrus CTRL codegen accepts at most one sync-wait per instruction;
    move extras onto preceding same-engine NOPs."""
    ctr = 0
    for fn in nc.m.functions:
        for bb in fn.blocks:
            newlist = []
            for ins in bb.instructions:
                si = ins.sync_info
                if si is not None and si.on_wait and len(si.on_wait) > max_waits:
                    waits = list(si.on_wait)
                    keep = waits[-max_waits:]
                    extra = waits[:-max_waits]
                    for k in range(0, len(extra), max_waits):
                        ctr += 1
                        nop = mybir.InstNoOp(name=f"SplitW-{ctr}", ins=[], outs=[])
                        nop.engine = ins.engine
                        nop.sync_info = mybir.SyncInfo(
                            on_wait=extra[k:k + max_waits], on_update=[])
                        newlist.append(nop)
                    ins.sync_info = mybir.SyncInfo(
                        on_wait=keep, on_update=list(si.on_update))
                newlist.append(ins)
            bb.instructions = newlist
    return ctr


def _gate_chain(nc, pool, la_t, u_t, shape, scale):
    """Return fp32 tile gate = sigmoid((la + log(u) - log1p(-u)) * scale).
    la_t/u_t are fp32 SBUF tiles of `shape`."""
    p, f = shape
    lg_u = pool.tile([p, f], F32, tag="gate_lgu")
    nc.scalar.activation(out=lg_u[:], in_=u_t[:], func=AF.Ln)
    lg_m = pool.tile([p, f], F32, tag="gate_lgm")
    # log(1 - u) = Ln(u * -1 + 1)
    nc.scalar.activation(out=lg_m[:], in_=u_t[:], func=AF.Ln, scale=-1.0, bias=1.0)
    d = pool.tile([p, f], F32, tag="gate_d")
    nc.vector.tensor_tensor(out=d[:], in0=lg_u[:], in1=lg_m[:], op=ALU.subtract)
    arg = pool.tile([p, f], F32, tag="gate_arg")
    nc.vector.tensor_tensor(out=arg[:], in0=d[:], in1=la_t[:], op=ALU.add)
    gate = pool.tile([p, f], F32, tag="gate_out")
    nc.scalar.activation(out=gate[:], in_=arg[:], func=AF.Sigmoid, scale=scale)
    return gate


def build_kernel(cfg: Cfg, split_waits: bool = True, repeat: int = 1):
    nc = bass.Bass("TRN2", target_bir_lowering=False, debug=False,
                   num_devices=cfg.n_cores)
    N, D, H = cfg.N, cfg.D, cfg.H
    KT = D // P  # k-tiles per projection (2 for D=256)

    # ---- I/O ----
    x_in = nc.dram_tensor("x", [N, D], F32, kind="ExternalInput")
    w1_in = nc.dram_tensor("w1", [2 * D, H], F32, kind="ExternalInput")
    b1_in = nc.dram_tensor("b1", [H], F32, kind="ExternalInput")
    w2_in = nc.dram_tensor("w2", [H, 1], F32, kind="ExternalInput")
    b2_in = nc.dram_tensor("b2", [1], F32, kind="ExternalInput")
    ident_in = nc.dram_tensor("ident", [P, P], F32, kind="ExternalInput")
    idx_i_in = nc.dram_tensor("idx_i", [P, cfg.SI], I16, kind="ExternalInput")
    idx_j_in = nc.dram_tensor("idx_j", [P, cfg.SI], I16, kind="ExternalInput")
    ela_in = nc.dram_tensor("ela", [P, cfg.NBLK], F32, kind="ExternalInput")
    u_in = nc.dram_tensor("u", [P, cfg.NBLK], F32, kind="ExternalInput")
    wla1_in = nc.dram_tensor("wla1c", [cfg.wg1_rows, cfg.wg1_cols], F32,
                             kind="ExternalInput")
    uw1_in = nc.dram_tensor("uw1c", [cfg.wg1_rows, cfg.wg1_cols], F32,
                            kind="ExternalInput")
    wla2_in = nc.dram_tensor("wla2c", [cfg.wg2_rows, cfg.wg2_cols], F32,
                             kind="ExternalInput")
    uw2_in = nc.dram_tensor("uw2c", [cfg.wg2_rows, cfg.wg2_cols], F32,
                            kind="ExternalInput")

    eg_out = nc.dram_tensor("eg", [P, cfg.NBLK], F32, kind="ExternalOutput")
    wg1_out = nc.dram_tensor("wg1c", [cfg.wg1_rows, cfg.wg1_cols], F32,
                             kind="ExternalOutput")
    wg2_out = nc.dram_tensor("wg2c", [cfg.wg2_rows, cfg.wg2_cols], F32,
                             kind="ExternalOutput")

    # fp16 node-projection table: row n = [P'(n) | Q(n)], 256 B
    r_dram = nc.dram_tensor("r_table", [cfg.RROWS, 2 * H], F16, kind="Internal")

    inv_beta = 1.0 / cfg.beta

    with tile.TileContext(nc) as tc:
        with (
            tc.tile_pool(name="singles", bufs=1) as singles,
            tc.tile_pool(name="pre", bufs=3) as pre,
            tc.tile_pool(name="pre_ps", bufs=2, space="PSUM") as pre_ps,
            tc.tile_pool(name="pq_ps", bufs=2, space="PSUM") as pq_ps,
            tc.tile_pool(name="gather", bufs=3) as gather,
            tc.tile_pool(name="hwork", bufs=3) as hwork,
            tc.tile_pool(name="sim_ps", bufs=1, space="PSUM") as sim_ps_pool,
            tc.tile_pool(name="gate", bufs=2) as gate_pool,
        ):
            # ---- constants ----
            nc.gpsimd.load_library(MLP_LIB)
            ident = singles.tile([P, P], F32)
            nc.sync.dma_start(out=ident[:], in_=ident_in.ap())
            w1_sb = singles.tile([P, KT, 2, H], F32)
            for kt in range(KT):
                nc.sync.dma_start(
                    out=w1_sb[:, kt, :, :],
                    in_=bass.AP(tensor=w1_in, offset=kt * P * H,
                                ap=[[H, P], [D * H, 2], [1, H]]),
                )
            b1_bc = singles.tile([P, H], F32)
            nc.sync.dma_start(
                out=b1_bc[:],
                in_=bass.AP(tensor=b1_in, offset=0, ap=[[0, P], [1, H]]),
            )
            w2_f32 = singles.tile([H, 1], F32)
            nc.sync.dma_start(out=w2_f32[:], in_=w2_in.ap())
            w2_f16 = singles.tile([H, 1], F16)
            nc.vector.tensor_copy(out=w2_f16[:], in_=w2_f32[:])
            b2_bc = singles.tile([P, 1], F32)
            nc.sync.dma_start(
                out=b2_bc[:],
                in_=bass.AP(tensor=b2_in, offset=0, ap=[[0, P], [1, 1]]),
            )
            idx_i_sb = singles.tile([P, cfg.SI], I16)
            nc.sync.dma_start(out=idx_i_sb[:], in_=idx_i_in.ap())
            idx_j_sb = singles.tile([P, cfg.SI], I16)
            nc.sync.dma_start(out=idx_j_sb[:], in_=idx_j_in.ap())
            w1_f16 = singles.tile([P, KT, 2 * H], F16)
            for kt in range(KT):
                nc.scalar.copy(out=w1_f16[:, kt, :], in_=w1_sb[:, kt, :, :])

            # ---- phases 1-4 (repeated `repeat` times for timing runs) ----
            for _rep in range(repeat):
                _run_phases(nc, cfg, locals())

    lower_extended_insts(nc)
    if split_waits:
        split_sync_waits(nc)
    return nc


def _run_phases(nc, cfg, env):
    (singles, pre, pre_ps, pq_ps, gather, hwork, sim_ps_pool, gate_pool, ident,
     w1_f16, b1_bc, w2_f16, b2_bc, idx_i_sb, idx_j_sb, r_dram, x_in, ela_in,
     u_in, eg_out, inv_beta, N, D, H, KT, wla1_in, uw1_in, wla2_in, uw2_in,
     wg1_out, wg2_out) = (
        env["singles"], env["pre"], env["pre_ps"], env["pq_ps"], env["gather"],
        env["hwork"], env["sim_ps_pool"], env["gate_pool"], env["ident"],
        env["w1_f16"], env["b1_bc"], env["w2_f16"], env["b2_bc"],
        env["idx_i_sb"], env["idx_j_sb"], env["r_dram"], env["x_in"],
        env["ela_in"], env["u_in"], env["eg_out"], env["inv_beta"], env["N"],
        env["D"], env["H"], env["KT"], env["wla1_in"], env["uw1_in"],
        env["wla2_in"], env["uw2_in"], env["wg1_out"], env["wg2_out"])
    if True:
            # ---- phase 1: build R table ----
            for c in range(cfg.NCHUNK):
                n0 = c * P
                cnt = min(P, N - n0)
                x_t = pre.tile([P, D], F32, tag="x_t")
                if cnt < P:
                    nc.vector.memset(x_t[:], 0.0)
                nc.sync.dma_start(out=x_t[:cnt, :], in_=x_in.ap()[n0:n0 + cnt, :])

                pq_psum = pq_ps.tile([P, 2 * H], F32, tag="pq")
                for kt in range(KT):
                    xt_ps = pre_ps.tile([P, P], F32, tag="xt_ps")
                    nc.tensor.transpose(
                        out=xt_ps[:], in_=x_t[:, kt * P:(kt + 1) * P],
                        identity=ident[:])
                    xt_sb = pre.tile([P, P], F16, tag="xt_sb")
                    nc.vector.tensor_copy(out=xt_sb[:], in_=xt_ps[:])
                    nc.tensor.matmul(
                        pq_psum[:], lhsT=xt_sb[:], rhs=w1_f16[:, kt, :],
                        start=(kt == 0), stop=(kt == KT - 1))

                r_sb = pre.tile([P, 2 * H], F16, tag="r_sb")
                # P' half: add b1 while casting to fp16
                nc.vector.tensor_tensor(
                    out=r_sb[:, 0:H], in0=pq_psum[:, 0:H], in1=b1_bc[:],
                    op=ALU.add)
                nc.vector.tensor_copy(out=r_sb[:, H:2 * H], in_=pq_psum[:, H:2 * H])
                nc.sync.dma_start(
                    out=r_dram.ap()[n0:n0 + P, :], in_=r_sb[:])

            # zero the pad rows after NPAD (offset gather may touch row NPAD)
            zpad = pre.tile([P, 2 * H], F16, tag="r_sb")
            nc.vector.memset(zpad[:], 0.0)
            nc.sync.dma_start(out=r_dram.ap()[cfg.NPAD:cfg.NPAD + P, :], in_=zpad[:])

            # ---- phase 2: edge pipeline ----
            sim_psum = sim_ps_pool.tile([P, cfg.NBLK], F32)
            gat_src_i = bass.AP(tensor=r_dram, offset=0,
                                ap=[[2 * H, cfg.NPAD], [1, 2 * H]])
            gat_src_j = bass.AP(tensor=r_dram, offset=H,
                                ap=[[2 * H, cfg.NPAD], [1, 2 * H]])

            for bt in range(cfg.NBATCH):
                s0 = bt * (cfg.BG // 16)
                g_i = gather.tile([P, 1, cfg.BG], F16, tag="g_i")
                nc.gpsimd.dma_gather(
                    out_ap=g_i[:], in_ap=gat_src_i,
                    idxs_ap=idx_i_sb[:, s0:s0 + cfg.BG // 16],
                    num_idxs=cfg.BG, num_idxs_reg=cfg.BG,
                    elem_size=2 * H, elem_step=2 * H, transpose=True,
                    single_packet=False)
                g_j = gather.tile([P, 1, cfg.BG], F16, tag="g_j")
                nc.gpsimd.dma_gather(
                    out_ap=g_j[:], in_ap=gat_src_j,
                    idxs_ap=idx_j_sb[:, s0:s0 + cfg.BG // 16],
                    num_idxs=cfg.BG, num_idxs_reg=cfg.BG,
                    elem_size=2 * H, elem_step=2 * H, transpose=True,
                    single_packet=False)

                h_t = hwork.tile([H, cfg.BG], F16, tag="h_t")
                nc.vector.tensor_tensor(
                    out=h_t[:], in0=g_i[0:H, 0, :], in1=g_j[0:H, 0, :],
                    op=ALU.add)
                nc.vector.tensor_scalar_max(out=h_t[:], in0=h_t[:], scalar1=0.0)

                for s in range(cfg.BLK_PER_BATCH):
                    col = bt * cfg.BLK_PER_BATCH + s
                    nc.tensor.matmul(
                        sim_psum[:, col:col + 1],
                        lhsT=h_t[:, s * P:(s + 1) * P], rhs=w2_f16[:],
                        start=True, stop=True)

            # ---- phase 3: edge gates ----
            ela_t = gate_pool.tile([P, cfg.NBLK], F32, tag="ela")
            nc.sync.dma_start(out=ela_t[:], in_=ela_in.ap())
            u_t = gate_pool.tile([P, cfg.NBLK], F32, tag="u")
            nc.sync.dma_start(out=u_t[:], in_=u_in.ap())
            gate = _gate_chain(nc, gate_pool, ela_t, u_t, (P, cfg.NBLK),
                               inv_beta)
            eg_t = gate_pool.tile([P, cfg.NBLK], F32, tag="eg")
            # eg = (sim + b2) * gate
            nc.vector.scalar_tensor_tensor(
                out=eg_t[:], in0=sim_psum[:],
                scalar=b2_bc[:, 0:1], in1=gate[:],
                op0=ALU.add, op1=ALU.mult)
            nc.sync.dma_start(out=eg_out.ap(), in_=eg_t[:])

            # ---- phase 4: weight gates ----
            for la_in, uu_in, oo, rows, cols in (
                (wla1_in, uw1_in, wg1_out, cfg.wg1_rows, cfg.wg1_cols),
                (wla2_in, uw2_in, wg2_out, cfg.wg2_rows, cfg.wg2_cols),
            ):
                la_t = gate_pool.tile([rows, cols], F32, tag="wla")
                nc.sync.dma_start(out=la_t[:], in_=la_in.ap())
                uu_t = gate_pool.tile([rows, cols], F32, tag="wu")
                nc.sync.dma_start(out=uu_t[:], in_=uu_in.ap())
                g_t = _gate_chain(nc, gate_pool, la_t, uu_t, (rows, cols),
                                  inv_beta)
                nc.sync.dma_start(out=oo.ap(), in_=g_t[:])


# ---------------- host side ----------------

N, E, D, H = 10000, 320000, 256, 64
N_CORES = 8
E_CORE = E // N_CORES            # 40000
REAL_CFG = Cfg(n_nodes=N, epad=40960, BG=5120, n_cores=N_CORES)

_nc_cache = {}


def _get_nc(cfg):
    key = id(cfg)
    if key not in _nc_cache:
        _nc_cache[key] = build_kernel(cfg)
    return _nc_cache[key]


def _wrap_idx(idx, si):
    """int16 indices -> [128, si] wrapped (16 partitions, replicated x8)."""
    idx = idx.astype(np.int16)
    w = idx.reshape(si, 16).T                      # [16, si]
    return np.tile(w, (8, 1)).copy()               # [128, si]


def make_in_maps(cfg, x, edge_index, edge_log_alpha, W1, b1, W2, b2,
                 wla1, wla2, u_edge, u_w1, u_w2):
    n_cores = cfg.n_cores
    e_core = edge_index.shape[1] // n_cores
    wg1_flat = wla1.reshape(n_cores, -1)
    uw1_flat = u_w1.reshape(n_cores, -1)
    wg2_flat = wla2.reshape(n_cores, -1)
    uw2_flat = u_w2.reshape(n_cores, -1)
    in_maps = []
    for c in range(n_cores):
        sl = slice(c * e_core, (c + 1) * e_core)
        ii = np.zeros(cfg.EPAD, np.int64)
        jj = np.zeros(cfg.EPAD, np.int64)
        ii[:e_core] = edge_index[0, sl]
        jj[:e_core] = edge_index[1, sl]
        ela = np.zeros(cfg.EPAD, np.float32)
        ela[:e_core] = edge_log_alpha[sl]
        uu = np.full(cfg.EPAD, 0.5, np.float32)
        uu[:e_core] = u_edge[sl]
        in_maps.append({
            "x": np.ascontiguousarray(x),
            "ident": np.eye(P, dtype=np.float32),
            "w1": np.ascontiguousarray(W1),
            "b1": np.ascontiguousarray(b1),
            "w2": np.ascontiguousarray(W2),
            "b2": np.ascontiguousarray(b2),
            "idx_i": _wrap_idx(ii, cfg.SI),
            "idx_j": _wrap_idx(jj, cfg.SI),
            "ela": np.ascontiguousarray(ela.reshape(cfg.NBLK, P).T),
            "u": np.ascontiguousarray(uu.reshape(cfg.NBLK, P).T),
            "wla1c": wg1_flat[c].reshape(cfg.wg1_rows, cfg.wg1_cols).copy(),
            "uw1c": uw1_flat[c].reshape(cfg.wg1_rows, cfg.wg1_cols).copy(),
            "wla2c": wg2_flat[c].reshape(cfg.wg2_rows, cfg.wg2_cols).copy(),
            "uw2c": uw2_flat[c].reshape(cfg.wg2_rows, cfg.wg2_cols).copy(),
        })
    return in_maps


def assemble(cfg, results, e_total, wla1_shape, wla2_shape):
    n_cores = cfg.n_cores
    e_core = e_total // n_cores
    eg = np.concatenate(
        [r["eg"].T.reshape(-1)[:e_core] for r in results])
    wg1 = np.concatenate([r["wg1c"].reshape(-1) for r in results]).reshape(
        wla1_shape)
    wg2 = np.concatenate([r["wg2c"].reshape(-1) for r in results]).reshape(
        wla2_shape)
    return eg, wg1, wg2


def kernel(x, edge_index, edge_log_alpha, W1, b1, W2, b2, wla1, wla2,
           u_edge, u_w1, u_w2):
    from concourse.bass_utils import run_bass_kernel_spmd

    cfg = REAL_CFG
    nc = _get_nc(cfg)
    in_maps = make_in_maps(
        cfg, np.asarray(x), np.asarray(edge_index),
        np.asarray(edge_log_alpha), np.asarray(W1), np.asarray(b1),
        np.asarray(W2), np.asarray(b2), np.asarray(wla1), np.asarray(wla2),
        np.asarray(u_edge), np.asarray(u_w1), np.asarray(u_w2))
    res = run_bass_kernel_spmd(nc, in_maps, core_ids=list(range(cfg.n_cores)))
    eg, wg1, wg2 = assemble(cfg, res.results, E, wla1.shape, wla2.shape)
    return eg.astype(np.float32), wg1.astype(np.float32), wg2.astype(np.float32)
